# revision 11
# baseline (speedup 1.0000x reference)
"""Trainium2 Bass kernel for nn_EnhancedMultiHeadDINAttention.

Math (algebraically collapsed from the reference):
  q = cand @ Wq + bq                          [B, D]
  scores[b,s] = x[b,s,:] . r_b + c_b          r = (Wk @ q)/scale, c = (q.bk)/scale
  scores -> mask(-1e9) -> * decay -> softmax -> w
  xw[b,:]  = sum_s w[b,s] x[b,s,:]
  xs[b,:]  = mean_s x[b,s,:]
  attended = xw @ Wv + bv + cand
  kbar     = xs @ Wk + bk
  inter_mean = concat([q, kbar, q-kbar, q*kbar]) @ Wi + bi
  out = concat([attended, inter_mean], -1)    [B, 2D]

Sharding: pure data parallel over batch, 8 cores x 64 batches.
x streams in as bf16 (cast in the DMA), kept in natural layout (s on
partitions) for the s-contraction and xbar-block-transposed (SBUF->SBUF
DMA transpose) for the d-contraction; both contractions run on the PE
with per-batch masked stationaries so 32 batches share one PSUM tile.
"""

import numpy as np

import concourse.bass as bass
import concourse.bacc as bacc
import concourse.tile as tile
from concourse import mybir
from concourse.masks import make_identity

F32 = mybir.dt.float32
BF16 = mybir.dt.bfloat16
I32 = mybir.dt.int32

N_CORES = 8
B, S, D = 512, 200, 256
BS = B // N_CORES          # 64 batches per core
S0 = 128                   # first s-half rows
S1 = S - S0                # 72 valid rows in second half
S1P = 80                   # padded to multiple of 16 for xbar transpose
NG = 8                     # x-load groups
GB = BS // NG              # 8 batches per load group
SCALE = 1.0 / np.sqrt(D // 4)   # 1/8
TIME_DECAY = 0.01
NEG_INF = -1e9

AF = mybir.ActivationFunctionType
ALU = mybir.AluOpType
AX = mybir.AxisListType


def _ap(ap, off, dims):
    return bass.AP(tensor=ap.tensor, offset=ap.offset + off, ap=dims)


def build_program():
    nc = bacc.Bacc("TRN2", target_bir_lowering=False, debug=False)

    past = nc.dram_tensor("past_interactions", [BS, S, D], F32, kind="ExternalInput").ap()
    cand = nc.dram_tensor("candidate_embedding", [BS, D], F32, kind="ExternalInput").ap()
    maskt = nc.dram_tensor("past_mask", [BS, S], I32, kind="ExternalInput").ap()
    tstamp = nc.dram_tensor("past_timestamps", [BS, S], F32, kind="ExternalInput").ap()
    Wq = nc.dram_tensor("Wq", [D, D], F32, kind="ExternalInput").ap()
    bq = nc.dram_tensor("bq", [D], F32, kind="ExternalInput").ap()
    Wk = nc.dram_tensor("Wk", [D, D], F32, kind="ExternalInput").ap()
    bk = nc.dram_tensor("bk", [D], F32, kind="ExternalInput").ap()
    Wv = nc.dram_tensor("Wv", [D, D], F32, kind="ExternalInput").ap()
    bv = nc.dram_tensor("bv", [D], F32, kind="ExternalInput").ap()
    Wi = nc.dram_tensor("Wi", [4 * D, D], F32, kind="ExternalInput").ap()
    bi = nc.dram_tensor("bi", [D], F32, kind="ExternalInput").ap()
    out = nc.dram_tensor("out", [BS, 2 * D], F32, kind="ExternalOutput").ap()

    with tile.TileContext(nc) as tc:
        _build(nc, tc, past, cand, maskt, tstamp, Wq, bq, Wk, bk, Wv, bv, Wi, bi, out)
    nc.compile()
    return nc


def _build(nc, tc, past, cand, maskt, tstamp, Wq, bq, Wk, bk, Wv, bv, Wi, bi, out):
    from contextlib import ExitStack

    ctx = ExitStack()
    consts = ctx.enter_context(tc.tile_pool(name="consts", bufs=1))
    xpool = ctx.enter_context(tc.tile_pool(name="x", bufs=NG))
    sm = ctx.enter_context(tc.tile_pool(name="sm", bufs=1))
    pp = ctx.enter_context(tc.tile_pool(name="pp", bufs=2, space="PSUM"))
    psc = ctx.enter_context(tc.tile_pool(name="psc", bufs=2, space="PSUM"))
    pxw = ctx.enter_context(tc.tile_pool(name="pxw", bufs=2, space="PSUM"))

    mm = nc.tensor.matmul

    # ---------------- constants / small loads ----------------
    ident = consts.tile([128, 128], F32)
    make_identity(nc, ident[:])

    Wq_sb = consts.tile([128, 2, D], F32)
    nc.scalar.dma_start(Wq_sb[:], Wq.rearrange("(c p) j -> p c j", p=128))
    Wk_sb = consts.tile([128, 2, D], F32)
    nc.scalar.dma_start(Wk_sb[:], Wk.rearrange("(c p) j -> p c j", p=128))
    Wv_sb = consts.tile([128, 2, D], F32)
    nc.scalar.dma_start(Wv_sb[:], Wv.rearrange("(c p) j -> p c j", p=128))
    Wi_sb = consts.tile([128, 8, D], F32)
    nc.scalar.dma_start(Wi_sb[:], Wi.rearrange("(c p) j -> p c j", p=128))

    bq_row = consts.tile([1, D], F32)
    nc.scalar.dma_start(bq_row[:], bq.unsqueeze(0))
    bk_row = consts.tile([1, D], F32)
    nc.scalar.dma_start(bk_row[:], bk.unsqueeze(0))
    bv_row = consts.tile([1, D], F32)
    nc.scalar.dma_start(bv_row[:], bv.unsqueeze(0))
    bi_row = consts.tile([1, D], F32)
    nc.scalar.dma_start(bi_row[:], bi.unsqueeze(0))
    bk_col = consts.tile([128, 2], F32)
    nc.scalar.dma_start(bk_col[:], bk.rearrange("(c p) -> p c", p=128))

    ones_row = consts.tile([1, BS], F32)
    nc.vector.memset(ones_row[:], 1.0)
    ones_col = consts.tile([128, 1], BF16)
    nc.vector.memset(ones_col[:], 1.0)
    neg_col = consts.tile([BS, 1], F32)
    nc.vector.memset(neg_col[:], NEG_INF)

    cand_sb = consts.tile([BS, D], F32)
    nc.scalar.dma_start(cand_sb[:], cand)
    mask_i = consts.tile([BS, S], I32)
    nc.scalar.dma_start(mask_i[:], maskt)
    ts_sb = consts.tile([BS, S], F32)
    nc.scalar.dma_start(ts_sb[:], tstamp)

    # m0 = 1 - mask  (predicate for "masked out"), int32 (walrus requires int mask)
    m0_i = consts.tile([BS, S], I32)
    nc.vector.tensor_scalar(m0_i[:], mask_i[:], -1.0, 1.0, ALU.mult, ALU.add)
    decay = consts.tile([BS, S], F32)
    nc.scalar.activation(decay[:], ts_sb[:], AF.Exp, scale=-TIME_DECAY)

    # ---------------- q / r / c chain ----------------
    candT = consts.tile([128, 2, BS], F32)
    for dc in range(2):
        pt = pp.tile([128, BS], F32, tag="pt")
        nc.tensor.transpose(pt[:], cand_sb[:, dc * 128:(dc + 1) * 128], ident[0:BS, 0:BS])
        nc.scalar.copy(candT[:, dc, :], pt[:])

    # WkT[p, jc, dc, :]: WkT[jc*128+p, dc*128+u] = Wk[dc*128+u, jc*128+p]
    WkT = consts.tile([128, 2, 2, 128], F32)
    for dc in range(2):
        for jc in range(2):
            pt = pp.tile([128, 128], F32, tag="pt")
            nc.tensor.transpose(pt[:], Wk_sb[:, dc, jc * 128:(jc + 1) * 128], ident[:])
            nc.scalar.copy(WkT[:, jc, dc, :], pt[:])

    # qT [j(2 chunks), b] = Wq.T @ candT + bq
    qT = consts.tile([128, 2, BS], F32)
    for jc in range(2):
        pq = pp.tile([128, BS], F32, tag="pt")
        mm(pq[:], Wq_sb[:, 0, jc * 128:(jc + 1) * 128], candT[:, 0, :], start=True, stop=False)
        mm(pq[:], Wq_sb[:, 1, jc * 128:(jc + 1) * 128], candT[:, 1, :], start=False, stop=False)
        mm(pq[:], bq_row[0:1, jc * 128:(jc + 1) * 128], ones_row[:], start=False, stop=True)
        nc.scalar.copy(qT[:, jc, :], pq[:])

    # rTm [128, dc, b, m]: masked stationaries, col m = b%32 holds rT[:, dc, b]*SCALE
    rTm = consts.tile([128, 2, BS, 32], BF16)
    nc.gpsimd.memset(rTm[:], 0.0)
    rtm_pitch = rTm[:].ap[0][0]
    for dc in range(2):
        pr = pp.tile([128, BS], F32, tag="pt")
        mm(pr[:], WkT[:, 0, dc, :], qT[:, 0, :], start=True, stop=False)
        mm(pr[:], WkT[:, 1, dc, :], qT[:, 1, :], start=False, stop=True)
        for a in range(2):
            # dest elements (b=32a+j, m=j): offset dc*BS*32 + 1024*a + 33*j
            dst = _ap(rTm[:], dc * BS * 32 + 1024 * a, [[rtm_pitch, 128], [33, 32]])
            nc.scalar.mul(dst, pr[:, 32 * a:32 * (a + 1)], SCALE)

    # c [b, 1] = (q . bk) * SCALE
    c_sb = sm.tile([BS, 1], F32)
    pc = pp.tile([BS, 1], F32, tag="pt")
    mm(pc[:], qT[:, 0, :], bk_col[:, 0:1], start=True, stop=False)
    mm(pc[:], qT[:, 1, :], bk_col[:, 1:2], start=False, stop=True)
    nc.scalar.mul(c_sb[:], pc[:], SCALE)

    # ---------------- x stream: load (cast bf16) + block transpose ----------------
    xh0s, xh1s, xT0s, xT1s = [], [], [], []
    for g in range(NG):
        lo, hi = g * GB, (g + 1) * GB
        xh0 = xpool.tile([128, GB, D], BF16, tag="xh0")
        nc.gpsimd.dma_start(xh0[:], past[lo:hi, 0:S0, :].rearrange("b s d -> s b d"))
        xh1 = xpool.tile([S1P, GB, D], BF16, tag="xh1")
        nc.vector.memset(xh1[64:S1P, :, :], 0.0)  # zero the 72:80 pad rows
        nc.gpsimd.dma_start(xh1[0:S1, :, :], past[lo:hi, S0:S, :].rearrange("b s d -> s b d"))
        # xT0[p, 2*bl+dc, s] = x[b, s, dc*128+p]
        xT0 = xpool.tile([128, 2 * GB, 128], BF16, tag="xT0")
        nc.sync.dma_start(xT0[:], xh0[:], transpose=True)
        xT1 = xpool.tile([128, 2 * GB, S1P], BF16, tag="xT1")
        nc.sync.dma_start(xT1[:], xh1[:], transpose=True)
        xh0s.append(xh0)
        xh1s.append(xh1)
        xT0s.append(xT0)
        xT1s.append(xT1)

    # ---------------- per 32-batch half: scores -> softmax -> xw ----------------
    scores_sb = sm.tile([BS, S], F32)
    w_sb = sm.tile([BS, S], F32)
    mx_sb = sm.tile([BS, 1], F32)
    sums_sb = sm.tile([BS, 1], F32)
    rs_sb = sm.tile([BS, 1], F32)
    xw_sb = sm.tile([BS, D], F32)
    xsum_sb = sm.tile([BS, D], F32)

    for a in range(2):
        sl = slice(32 * a, 32 * (a + 1))
        # scores: accumulate all 32 batches into two psum tiles (one per s-half)
        ps0 = psc.tile([32, 128], F32, tag="ps0")
        ps1 = psc.tile([32, 80], F32, tag="ps1")
        for j in range(32):
            b = 32 * a + j
            g, bl = b // GB, b % GB
            o0 = ps0[:, :]
            o1 = ps1[:, :]
            mm(o0, rTm[:, 0, b, :], xT0s[g][:, 2 * bl + 0, :],
               start=(j == 0), stop=False)
            mm(o0, rTm[:, 1, b, :], xT0s[g][:, 2 * bl + 1, :],
               start=False, stop=(j == 31))
            mm(o1, rTm[:, 0, b, :], xT1s[g][:, 2 * bl + 0, :],
               start=(j == 0), stop=False)
            mm(o1, rTm[:, 1, b, :], xT1s[g][:, 2 * bl + 1, :],
               start=False, stop=(j == 31))
        # extract + add c
        nc.scalar.add(scores_sb[sl, 0:S0], ps0[:, :], c_sb[sl, 0:1])
        nc.scalar.add(scores_sb[sl, S0:S], ps1[:, 0:S1], c_sb[sl, 0:1])

        # masked softmax over s
        nc.vector.tensor_copy(w_sb[sl], scores_sb[sl])
        negb = _ap(neg_col[:], 32 * a * neg_col[:].ap[0][0], [[neg_col[:].ap[0][0], 32], [0, S]])
        nc.vector.copy_predicated(w_sb[sl], m0_i[sl], negb)
        nc.vector.tensor_mul(w_sb[sl], w_sb[sl], decay[sl])
        nc.vector.tensor_reduce(mx_sb[sl], w_sb[sl], axis=AX.X, op=ALU.max, negate=True)
        nc.scalar.activation(w_sb[sl], w_sb[sl], AF.Exp, bias=mx_sb[sl], scale=1.0,
                             accum_out=sums_sb[sl])
        nc.vector.reciprocal(rs_sb[sl], sums_sb[sl])
        nc.vector.tensor_scalar_mul(w_sb[sl], w_sb[sl], rs_sb[sl])

        # wT (s-major) then scatter into masked stationaries [s, 64]:
        # col j = w_b (b = 32a+j), col 32+j = ones
        wm0 = consts.tile([128, 32, 64], BF16, tag=f"wm0_{a}")
        nc.gpsimd.memset(wm0[:], 0.0)
        wm1 = consts.tile([S1, 32, 64], BF16, tag=f"wm1_{a}")
        nc.gpsimd.memset(wm1[:], 0.0)
        p0 = wm0[:].ap[0][0]
        p1 = wm1[:].ap[0][0]

        pw0 = pp.tile([128, 32], F32, tag="pt")
        nc.tensor.transpose(pw0[:], w_sb[sl, 0:128], ident[sl, sl])
        nc.scalar.copy(_ap(wm0[:], 0, [[p0, 128], [65, 32]]), pw0[:])
        nc.scalar.copy(_ap(wm0[:], 32, [[p0, 128], [65, 32]]),
                       _ap(ones_col[:], 0, [[ones_col[:].ap[0][0], 128], [0, 32]]))
        pw1 = pp.tile([S1, 32], F32, tag="pt")
        nc.tensor.transpose(pw1[:], w_sb[sl, 128:S], ident[sl, sl])
        nc.scalar.copy(_ap(wm1[:], 0, [[p1, S1], [65, 32]]), pw1[:])
        nc.scalar.copy(_ap(wm1[:], 32, [[p1, S1], [65, 32]]),
                       _ap(ones_col[:], 0, [[ones_col[:].ap[0][0], S1], [0, 32]]))

        # xw / xsum: rows 64a + j -> xw_b, rows 64a + 32 + j -> xsum_b ... single
        # [128, 256] psum tile per half: rows [0:32] xw, [32:64] xsum (a=0) etc.
        px = pxw.tile([64, D], F32, tag="px")
        for j in range(32):
            b = 32 * a + j
            g, bl = b // GB, b % GB
            mm(px[:, :], wm0[:, j, :], xh0s[g][:, bl, :],
               start=(j == 0), stop=False, tile_position=(0, 0))
            mm(px[:, :], wm1[:, j, :], xh1s[g][0:S1, bl, :],
               start=False, stop=(j == 31), tile_position=(0, 0))
        nc.scalar.copy(xw_sb[sl, :], px[0:32, :])
        nc.scalar.mul(xsum_sb[sl, :], px[32:64, :], 1.0 / S)

    # ---------------- finals ----------------
    xwT = consts.tile([128, 2, BS], F32)
    xsT = consts.tile([128, 2, BS], F32)
    for dc in range(2):
        pt = pp.tile([128, BS], F32, tag="pt")
        nc.tensor.transpose(pt[:], xw_sb[:, dc * 128:(dc + 1) * 128], ident[0:BS, 0:BS])
        nc.scalar.copy(xwT[:, dc, :], pt[:])
        pt2 = pp.tile([128, BS], F32, tag="pt")
        nc.tensor.transpose(pt2[:], xsum_sb[:, dc * 128:(dc + 1) * 128], ident[0:BS, 0:BS])
        nc.scalar.copy(xsT[:, dc, :], pt2[:])

    # kbarT [d(2 chunks), b] = Wk.T @ xsT + bk
    kT = consts.tile([128, 2, BS], F32)
    for dc in range(2):
        pk = pp.tile([128, BS], F32, tag="pt")
        mm(pk[:], Wk_sb[:, 0, dc * 128:(dc + 1) * 128], xsT[:, 0, :], start=True, stop=False)
        mm(pk[:], Wk_sb[:, 1, dc * 128:(dc + 1) * 128], xsT[:, 1, :], start=False, stop=False)
        mm(pk[:], bk_row[0:1, dc * 128:(dc + 1) * 128], ones_row[:], start=False, stop=True)
        nc.scalar.copy(kT[:, dc, :], pk[:])

    fd = consts.tile([128, 2, BS], F32)
    fm = consts.tile([128, 2, BS], F32)
    for dc in range(2):
        nc.vector.tensor_sub(fd[:, dc, :], qT[:, dc, :], kT[:, dc, :])
        nc.vector.tensor_mul(fm[:, dc, :], qT[:, dc, :], kT[:, dc, :])

    # attended = xw @ Wv + bv + cand
    pa = pp.tile([BS, D], F32, tag="pt")
    mm(pa[:], xwT[:, 0, :], Wv_sb[:, 0, :], start=True, stop=False)
    mm(pa[:], xwT[:, 1, :], Wv_sb[:, 1, :], start=False, stop=False)
    mm(pa[:], ones_row[:], bv_row[:], start=False, stop=True)
    att_sb = sm.tile([BS, D], F32)
    nc.vector.tensor_add(att_sb[:], pa[:], cand_sb[:])
    nc.sync.dma_start(out[:, 0:D], att_sb[:])

    # inter_mean = feat @ Wi + bi
    pm = pp.tile([BS, D], F32, tag="pt")
    feat_chunks = [qT[:, 0, :], qT[:, 1, :], kT[:, 0, :], kT[:, 1, :],
                   fd[:, 0, :], fd[:, 1, :], fm[:, 0, :], fm[:, 1, :]]
    for ci, fc in enumerate(feat_chunks):
        mm(pm[:], fc, Wi_sb[:, ci, :], start=(ci == 0), stop=False)
    mm(pm[:], ones_row[:], bi_row[:], start=False, stop=True)
    mi_sb = sm.tile([BS, D], F32)
    nc.scalar.copy(mi_sb[:], pm[:])
    nc.sync.dma_start(out[:, D:2 * D], mi_sb[:])

    ctx.close()


_NC_CACHE = None


def _get_program():
    global _NC_CACHE
    if _NC_CACHE is None:
        _NC_CACHE = build_program()
    return _NC_CACHE


def _shard_inputs(inputs):
    in_maps = []
    for i in range(N_CORES):
        lo, hi = i * BS, (i + 1) * BS
        in_maps.append({
            "past_interactions": np.ascontiguousarray(inputs["past_interactions"][lo:hi]).astype(np.float32, copy=False),
            "candidate_embedding": np.ascontiguousarray(inputs["candidate_embedding"][lo:hi]).astype(np.float32, copy=False),
            "past_mask": np.ascontiguousarray(inputs["past_mask"][lo:hi]).astype(np.int32, copy=False),
            "past_timestamps": np.ascontiguousarray(inputs["past_timestamps"][lo:hi]).astype(np.float32, copy=False),
            "Wq": np.asarray(inputs["Wq"], np.float32),
            "bq": np.asarray(inputs["bq"], np.float32),
            "Wk": np.asarray(inputs["Wk"], np.float32),
            "bk": np.asarray(inputs["bk"], np.float32),
            "Wv": np.asarray(inputs["Wv"], np.float32),
            "bv": np.asarray(inputs["bv"], np.float32),
            "Wi": np.asarray(inputs["Wi"], np.float32),
            "bi": np.asarray(inputs["bi"], np.float32),
        })
    return in_maps


def run(inputs, trace=False):
    from concourse.bass_utils import run_bass_kernel_spmd

    nc = _get_program()
    in_maps = _shard_inputs(inputs)
    res = run_bass_kernel_spmd(nc, in_maps, list(range(N_CORES)), trace=trace)
    outs = [res.results[i]["out"] for i in range(N_CORES)]
    full = np.concatenate(outs, axis=0).astype(np.float32)
    return full, res


def kernel(**inputs):
    inputs = {k: np.asarray(v) for k, v in inputs.items()}
    full, _ = run(inputs, trace=False)
    return full
